# revision 3
# baseline (speedup 1.0000x reference)
"""Sliding-window (banded) multi-head self-attention on 8 trn2 NeuronCores.

Sequence-parallel sharding: batch b, 2048 tokens -> 4 chunks of 512 queries;
core c handles batch c//4, chunk c%4.  Each core receives x^T for its 512
tokens plus a 128-token halo (zero-padded for chunk 0), computes
qkv projection + RoPE + banded attention (window 129) + out projection for
its rows, and returns [512, 2048].  No cross-core communication.

Layout choices (all matmuls contract over the partition dim):
  - x^T resident in SBUF as [128, 16(e-chunk), 640(tok)]
  - Q^T/K^T per head feature-major [128(d), tok] straight out of PSUM;
    RoPE pairs de-interleaved host-side (d' = evens then odds) so
    rotate_half is a partition-half swap (SBUF->SBUF DMA).
  - V token-major [128(tok), d] (natural for PV lhsT).
  - scores^T computed per k-chunk as [128(k), 256(q)] fp32r matmuls;
    exp on ACT; 0/1 band-mask multiply; PV + replicated-ones rowsum
    matmuls accumulate into one PSUM bank; normalize with reciprocal.
  - out projection accumulates 16 hd-chunks; bias added host-side.

Scheduling notes (v2):
  - All SBUF<->SBUF swap DMAs and OUT stores are issued from the ACT
    engine's HWDGE queue so the SP queue never blocks on data-dependent
    waits and weight prefetch streams continuously.
  - Startup: xt chunk DMAs are interleaved with WQ/WK of head 0 so the
    PE starts accumulating Q0 while x is still streaming in; group 0's
    V projection is emitted ec-outer (5 parallel PSUM accumulators
    borrowed from the then-idle attention pools) so it proceeds at DMA
    arrival pace instead of stalling on the full 4.2MB WV block.
"""

import math
import numpy as np

import concourse.bass as bass
import concourse.tile as tile
from concourse import mybir
from concourse.bass_utils import run_bass_kernel_spmd
from concourse.vector_clock import ScopedClock, VectorClock


def _legalize_single_wait(nc):
    """This walrus build accepts only ONE sync-wait per lowered command
    ("Too many sync wait commands").  Move all but the last wait of every
    instruction onto single-wait NoOps prepended on the same engine: engines
    are in-order, so stalling on the NoOps is equivalent.  SP-issued DMAs are
    gated the same way (descriptor push happens in SP program order)."""
    nid = [0]
    for f in nc.m.functions:
        for blk in f.blocks:
            out = []
            changed = False
            for inst in blk.instructions:
                si = inst.sync_info
                waits = list(si.on_wait) if si and si.on_wait else []
                if len(waits) > 1:
                    changed = True
                    for w in waits[:-1]:
                        nop = mybir.InstNoOp(name=f"waitnop-{nid[0]}", ins=[], outs=[])
                        nid[0] += 1
                        nop.engine = inst.engine
                        nop.sync_info = mybir.SyncInfo(on_wait=[w], on_update=[])
                        out.append(nop)
                    inst.sync_info = mybir.SyncInfo(
                        on_wait=[waits[-1]], on_update=list(si.on_update or [])
                    )
                out.append(inst)
            if changed:
                blk.instructions = out
    return nc


def _install_drain_split_patch():
    """Split TileContext's closing drain into single-wait drains: walrus's
    CTRL_NO command rejects the catch-all drain ("Too many sync waits")."""
    if getattr(tile.TileContext, "_drain_split_patched", False):
        return

    def _patched(self, tick_clock, wait_clock):
        gvc = tick_clock.global_clock  # VectorClock over the 27 procs
        n = len(gvc)
        procs = [i for i in range(n) if gvc[i] > 0]
        for pi in procs:
            vc = VectorClock([gvc[i] if i == pi else 0 for i in range(n)])
            d = self.nc.sync.drain()
            wait_clock.add_sem_waits(d.ins, ScopedClock({None: vc}))
        self.nc.all_engine_barrier()
        assert self.sems is not None
        popped = self.nc._tile_sem_poison_stack.pop()
        assert popped is self._sem_poison
        self.nc.clear_and_free_semaphores(list(self.sems.allocated().values()))
        self.nc.all_engine_barrier()

    tile.TileContext._drain_and_barrier = _patched
    tile.TileContext._drain_split_patched = True


_install_drain_split_patch()

EMBED = 2048
HEADS = 16
HD = 128
WINDOW = 128
THETA = 10000.0
B = 2
L = 2048
S = 512            # queries per core
T = S + WINDOW     # k/v tokens per core (incl halo)
NCORES = 8
P = 128
F32 = mybir.dt.float32
F32R = mybir.dt.float32r
BF16 = mybir.dt.bfloat16


def build_bass(legalize=True):
    nc = bass.Bass("TRN2", target_bir_lowering=False, debug=False)

    XT = nc.dram_tensor("XT", [P, EMBED // P, T], F32R, kind="ExternalInput")
    WQ = nc.dram_tensor("WQ", [HEADS, P, EMBED // P, HD], F32R, kind="ExternalInput")
    WK = nc.dram_tensor("WK", [HEADS, P, EMBED // P, HD], F32R, kind="ExternalInput")
    WV = nc.dram_tensor("WV", [4, P, EMBED // P, 512], F32R, kind="ExternalInput")
    WO = nc.dram_tensor("WO", [4, P, EMBED // P, 512], F32R, kind="ExternalInput")
    COSQ = nc.dram_tensor("COSQ", [P, S], F32R, kind="ExternalInput")
    SINQ = nc.dram_tensor("SINQ", [P, S], F32R, kind="ExternalInput")
    COSK = nc.dram_tensor("COSK", [P, T], F32R, kind="ExternalInput")
    SINK = nc.dram_tensor("SINK", [P, T], F32R, kind="ExternalInput")
    MASKS = nc.dram_tensor("MASKS", [4, P, 256], F32R, kind="ExternalInput")
    ONES = nc.dram_tensor("ONES", [P, P], F32R, kind="ExternalInput")
    OUT = nc.dram_tensor("OUT", [S, EMBED], F32, kind="ExternalOutput")

    EC = EMBED // P  # 16 e-chunks
    GROUPS = 4       # head groups of 4 (for V projection at N=512)
    GH = HEADS // GROUPS
    HT = T // 2      # 320

    with tile.TileContext(nc) as tc:
        with (
            tc.tile_pool(name="persist", bufs=1) as persist,
            tc.tile_pool(name="wbig", bufs=8) as wbig,
            tc.tile_pool(name="rope", bufs=2) as rope,
            tc.tile_pool(name="vsb", bufs=8) as vsb_pool,
            tc.tile_pool(name="attn", bufs=4) as attn_pool,
            tc.tile_pool(name="small", bufs=2) as small,
            tc.tile_pool(name="outsb", bufs=2) as outsb,
            tc.tile_pool(name="ps_qv", bufs=2, space="PSUM") as ps_qv,
            tc.tile_pool(name="ps_k", bufs=1, space="PSUM") as ps_k,
            tc.tile_pool(name="ps_sc", bufs=2, space="PSUM") as ps_sc,
            tc.tile_pool(name="ps_oc", bufs=2, space="PSUM") as ps_oc,
        ):
            # ---- persistent tiles ----
            xt = persist.tile([P, EC, T], F32R, tag="xt")
            cosq = persist.tile([P, S], F32R, tag="cosq")
            sinq = persist.tile([P, S], F32R, tag="sinq")
            cosk = persist.tile([P, T], F32R, tag="cosk")
            sink = persist.tile([P, T], F32R, tag="sink")
            masks = persist.tile([P, 4, 256], F32R, tag="masks")
            ones_full = persist.tile([P, P], F32R, tag="ones_full")
            out_norm = persist.tile([P, HEADS, S], F32R, tag="out_norm")

            def load_wq(h, name, fine=False):
                w = wbig.tile([P, EC, HD], F32R, tag="wbig", name=name)
                if fine:
                    return w  # caller interleaves the DMAs
                for i8 in range(2):
                    nc.sync.dma_start(
                        w[:, 8 * i8 : 8 * i8 + 8, :],
                        (WQ if name[1] == "q" else WK).ap()[
                            h, :, 8 * i8 : 8 * i8 + 8, :
                        ],
                    )
                return w

            def load_wv(g):
                wv_qs = []
                for q in range(4):
                    wvq = wbig.tile([P, 4, 512], F32R, tag="wbig", name=f"wv{g}_{q}")
                    nc.sync.dma_start(
                        wvq[:, 0:2, :], WV.ap()[g, :, 4 * q : 4 * q + 2, :]
                    )
                    nc.sync.dma_start(
                        wvq[:, 2:4, :], WV.ap()[g, :, 4 * q + 2 : 4 * q + 4, :]
                    )
                    wv_qs.append(wvq)
                return wv_qs

            # ---- startup DMA order: xt interleaved with wq0/wk0 ----
            wq0_sb = wbig.tile([P, EC, HD], F32R, tag="wbig", name="wq0")
            wk0_sb = wbig.tile([P, EC, HD], F32R, tag="wbig", name="wk0")
            for i4 in range(4):
                for ec in range(4 * i4, 4 * i4 + 4):
                    nc.sync.dma_start(xt[:, ec, :], XT.ap()[:, ec, :])
                nc.sync.dma_start(
                    wq0_sb[:, 4 * i4 : 4 * i4 + 4, :],
                    WQ.ap()[0, :, 4 * i4 : 4 * i4 + 4, :],
                )
                nc.sync.dma_start(
                    wk0_sb[:, 4 * i4 : 4 * i4 + 4, :],
                    WK.ap()[0, :, 4 * i4 : 4 * i4 + 4, :],
                )
            nc.sync.dma_start(cosq, COSQ.ap())
            nc.sync.dma_start(sinq, SINQ.ap())
            nc.sync.dma_start(cosk, COSK.ap())
            nc.sync.dma_start(sink, SINK.ap())

            def emit_qk_compute(h, wq_sb, wk_sb):
                # Q projection (feature-major [d, q]) + RoPE
                psq = ps_qv.tile([P, S], F32, tag="psqv", name=f"psq{h}")
                for ec in range(EC):
                    nc.tensor.matmul(
                        psq,
                        wq_sb[:, ec, :],
                        xt[:, ec, WINDOW:T],
                        start=(ec == 0),
                        stop=(ec == EC - 1),
                    )
                q_sb = rope.tile([P, S], F32R, tag="qrope", name=f"q{h}")
                qraw_t = rope.tile([P, T], F32R, tag="raw", name=f"qraw{h}")
                qraw = qraw_t[:, 0:S]
                nc.scalar.copy(qraw, psq)
                nc.vector.tensor_mul(q_sb, psq, cosq)
                qsw_t = rope.tile([P, T], F32R, tag="sw", name=f"qsw{h}")
                qsw = qsw_t[:, 0:S]
                nc.scalar.dma_start(qsw[0:64, :], qraw[64:128, :])
                nc.scalar.dma_start(qsw[64:128, :], qraw[0:64, :])
                nc.gpsimd.tensor_mul(qsw, qsw, sinq)
                nc.vector.tensor_add(q_sb, q_sb, qsw)

                # K projection + RoPE (tokens 0..640 in two 320 halves)
                psk1 = ps_k.tile([P, HT], F32, tag="psk1", name=f"psk1_{h}")
                psk2 = ps_k.tile([P, HT], F32, tag="psk2", name=f"psk2_{h}")
                for ec in range(EC):
                    nc.tensor.matmul(
                        psk1,
                        wk_sb[:, ec, :],
                        xt[:, ec, 0:HT],
                        start=(ec == 0),
                        stop=(ec == EC - 1),
                    )
                for ec in range(EC):
                    nc.tensor.matmul(
                        psk2,
                        wk_sb[:, ec, :],
                        xt[:, ec, HT:T],
                        start=(ec == 0),
                        stop=(ec == EC - 1),
                    )
                k_sb = rope.tile([P, T], F32R, tag="krope", name=f"k{h}")
                kraw = rope.tile([P, T], F32R, tag="raw", name=f"kraw{h}")
                nc.scalar.copy(kraw[:, 0:HT], psk1)
                nc.scalar.copy(kraw[:, HT:T], psk2)
                nc.vector.tensor_mul(k_sb[:, 0:HT], psk1, cosk[:, 0:HT])
                nc.vector.tensor_mul(k_sb[:, HT:T], psk2, cosk[:, HT:T])
                ksw = rope.tile([P, T], F32R, tag="sw", name=f"ksw{h}")
                nc.scalar.dma_start(ksw[0:64, :], kraw[64:128, :])
                nc.scalar.dma_start(ksw[64:128, :], kraw[0:64, :])
                nc.gpsimd.tensor_mul(ksw, ksw, sink)
                nc.vector.tensor_add(k_sb, k_sb, ksw)
                return q_sb, k_sb

            def emit_proj(h):
                wq_sb = load_wq(h, f"wq{h}")
                wk_sb = load_wq(h, f"wk{h}")
                return emit_qk_compute(h, wq_sb, wk_sb)

            def emit_vproj_steady(g, wv_qs):
                # weights already prefetched a group ahead: tt-outer is fine
                v_tiles = []
                for tt in range(T // P):  # 5 token tiles
                    psv = ps_qv.tile([P, 512], F32, tag="psqv", name=f"psv{g}_{tt}")
                    for ec in range(EC):
                        nc.tensor.matmul(
                            psv,
                            xt[:, ec, tt * P : (tt + 1) * P],
                            wv_qs[ec // 4][:, ec % 4, :],
                            start=(ec == 0),
                            stop=(ec == EC - 1),
                        )
                    v_sb = vsb_pool.tile([P, 512], F32R, tag="vsb", name=f"v{g}_{tt}")
                    nc.scalar.copy(v_sb, psv)
                    v_tiles.append(v_sb)
                return v_tiles

            def emit_vproj_g0(wv_qs):
                # ec-outer: 5 parallel accumulators so V proceeds at DMA pace.
                # Borrow banks from the (still idle) attention pools.
                psv = [
                    ps_oc.tile([P, 512], F32, tag="oc", name="psv0_0"),
                    ps_oc.tile([P, 512], F32, tag="oc", name="psv0_1"),
                    ps_sc.tile([P, 512], F32, tag="sc", name="psv0_2"),
                    ps_sc.tile([P, 512], F32, tag="sc", name="psv0_3"),
                    ps_qv.tile([P, 512], F32, tag="psqv", name="psv0_4"),
                ]
                for ec in range(EC):
                    for tt in range(T // P):
                        nc.tensor.matmul(
                            psv[tt],
                            xt[:, ec, tt * P : (tt + 1) * P],
                            wv_qs[ec // 4][:, ec % 4, :],
                            start=(ec == 0),
                            stop=(ec == EC - 1),
                        )
                v_tiles = []
                for tt in range(T // P):
                    v_sb = vsb_pool.tile([P, 512], F32R, tag="vsb", name=f"v0_{tt}")
                    nc.scalar.copy(v_sb, psv[tt])
                    v_tiles.append(v_sb)
                return v_tiles

            def emit_attn(h, q_sb, k_sb, v_tiles):
                hh = h % GH
                for p in range(2):
                    qs = p * 256
                    ets = []
                    for j in range(3):  # roles R1,R2,R3 -> k-chunk 2p+j
                        c = 2 * p + j
                        midx = 3 if (j == 0 and p == 1) else j
                        psc = ps_sc.tile([P, 256], F32, tag="sc", name=f"sc{h}_{p}{j}")
                        nc.tensor.matmul(
                            psc,
                            k_sb[:, c * P : (c + 1) * P],
                            q_sb[:, qs : qs + 256],
                            start=True,
                            stop=True,
                        )
                        et = attn_pool.tile([P, 256], F32R, tag="attn", name=f"et{h}_{p}{j}")
                        nc.scalar.activation(
                            et, psc, mybir.ActivationFunctionType.Exp
                        )
                        nc.vector.tensor_mul(et, et, masks[:, midx, :])
                        ets.append(et)

                    poc = ps_oc.tile([P, 512], F32, tag="oc", name=f"poc{h}_{p}")
                    for j in range(3):
                        c = 2 * p + j
                        nc.tensor.matmul(
                            poc[:, 0:256],
                            v_tiles[c][:, hh * HD : (hh + 1) * HD],
                            ets[j],
                            start=(j == 0),
                            stop=False,
                        )
                        nc.tensor.matmul(
                            poc[:, 256:512],
                            ones_full,
                            ets[j],
                            start=False,
                            stop=(j == 2),
                        )
                    recip = small.tile([P, 256], F32, tag="recip", name=f"rc{h}_{p}")
                    nc.vector.reciprocal(recip, poc[:, 256:512])
                    nc.vector.tensor_mul(
                        out_norm[:, h, qs : qs + 256], poc[:, 0:256], recip
                    )

            # ---- group 0: special trickle-friendly emission ----
            qk0 = emit_qk_compute(0, wq0_sb, wk0_sb)
            wv_g0 = load_wv(0)
            nc.sync.dma_start(masks, MASKS.ap().rearrange("m p q -> p m q"))
            nc.sync.dma_start(ones_full, ONES.ap())
            v_groups = {0: emit_vproj_g0(wv_g0)}
            pending = (0, qk0[0], qk0[1])
            for hh in range(1, GH):
                qk = emit_proj(hh)
                ph = pending[0]
                emit_attn(ph, pending[1], pending[2], v_groups[ph // GH])
                pending = (hh, qk[0], qk[1])

            # ---- groups 1..3: steady state ----
            for g in range(1, GROUPS):
                h0 = g * GH
                qk = emit_proj(h0)
                ph = pending[0]
                emit_attn(ph, pending[1], pending[2], v_groups[ph // GH])
                pending = (h0, qk[0], qk[1])
                v_groups[g] = emit_vproj_steady(g, load_wv(g))
                for hh in range(1, GH):
                    h = g * GH + hh
                    qk = emit_proj(h)
                    ph = pending[0]
                    emit_attn(ph, pending[1], pending[2], v_groups[ph // GH])
                    pending = (h, qk[0], qk[1])

            # prefetch first out-projection weight slices, then last attention
            def load_wo(eo):
                wo_qs = []
                for q in range(4):
                    woq = wbig.tile([P, 4, 512], F32R, tag="wbig", name=f"wo{eo}_{q}")
                    nc.sync.dma_start(
                        woq[:, 0:2, :], WO.ap()[eo, :, 4 * q : 4 * q + 2, :]
                    )
                    nc.sync.dma_start(
                        woq[:, 2:4, :], WO.ap()[eo, :, 4 * q + 2 : 4 * q + 4, :]
                    )
                    wo_qs.append(woq)
                return wo_qs

            wo_qs = load_wo(0)
            ph = pending[0]
            emit_attn(ph, pending[1], pending[2], v_groups[ph // GH])

            # ---- out projection: OUT[t, e] = sum_hd out_norm^T . WO ----
            for eo in range(4):
                e0 = eo * 512
                if eo > 0:
                    wo_qs = load_wo(eo)
                for tt in range(4):
                    pso = ps_oc.tile([P, 512], F32, tag="oc")
                    for hd in range(HEADS):
                        nc.tensor.matmul(
                            pso,
                            out_norm[:, hd, tt * P : (tt + 1) * P],
                            wo_qs[hd // 4][:, hd % 4, :],
                            start=(hd == 0),
                            stop=(hd == HEADS - 1),
                        )
                    o_sb = outsb.tile([P, 512], F32, tag="osb")
                    nc.scalar.copy(o_sb, pso)
                    nc.scalar.dma_start(
                        OUT.ap()[tt * P : (tt + 1) * P, e0 : e0 + 512], o_sb
                    )

    if legalize:
        _legalize_single_wait(nc)
    return nc


def _rope_tables(pos, scale):
    """Feature-major [128, len(pos)] cos / sin' tables in de-interleaved d order.

    cos'[i, t] = cos(pos_t * invf[i % 64]) ; sin'[0:64] = -sin, sin'[64:128] = +sin.
    """
    inv_freq = 1.0 / (THETA ** (np.arange(0, HD, 2, dtype=np.float64) / HD))  # [64]
    ang = pos[None, :] * inv_freq[:, None]  # [64, T]
    cos = np.cos(ang)
    sin = np.sin(ang)
    cos_t = np.concatenate([cos, cos], axis=0) * scale
    sin_t = np.concatenate([-sin, sin], axis=0) * scale
    return cos_t.astype(np.float32), sin_t.astype(np.float32)


def _band_masks(start):
    """[4, 128, 256] multiplicative masks.

    Element (m, kp, qf): role m in {R1 pair0, R2, R3, R1 pair1};
    local key j = c*128 + kp, local query r = qs + qf;
    valid iff r <= j <= r + 128 and (global key) start - 128 + j >= 0.
    """
    out = np.zeros((4, P, 256), dtype=np.float32)
    roles = [(0, 0), (1, 0), (2, 0), (2, 256)]  # (chunk c, query offset qs)
    for m, (c, qs) in enumerate(roles):
        kp = np.arange(P)[:, None]
        qf = np.arange(256)[None, :]
        j = c * P + kp
        r = qs + qf
        valid = (r <= j) & (j <= r + WINDOW) & (start - WINDOW + j >= 0)
        out[m] = valid.astype(np.float32)
    return out


_CACHED = {}
LAST_RESULT = {}


def prepare_in_maps(x, W_qkv, W_out, b_out):
    x = np.asarray(x, dtype=np.float32)
    W_qkv = np.asarray(W_qkv, dtype=np.float32)
    W_out = np.asarray(W_out, dtype=np.float32)

    # host-side weight layout prep
    perm = np.concatenate([np.arange(0, HD, 2), np.arange(1, HD, 2)])  # de-interleave
    w4 = W_qkv.reshape(EMBED, HEADS, HD, 3)
    # [h, e, d] -> [h, p, ec, d] partition-major contiguous
    WQ = w4[..., 0].transpose(1, 0, 2)[:, :, perm].reshape(HEADS, EMBED // P, P, HD)
    WQ = np.ascontiguousarray(WQ.transpose(0, 2, 1, 3))
    WK = w4[..., 1].transpose(1, 0, 2)[:, :, perm].reshape(HEADS, EMBED // P, P, HD)
    WK = np.ascontiguousarray(WK.transpose(0, 2, 1, 3))
    # [e, f] -> [g, p, ec, 512]
    WV = w4[..., 2].reshape(EMBED // P, P, 4, 512)
    WV = np.ascontiguousarray(WV.transpose(2, 1, 0, 3))
    WOa = W_out.reshape(EMBED // P, P, 4, 512)
    WOa = np.ascontiguousarray(WOa.transpose(2, 1, 0, 3))

    in_maps = []
    for core in range(NCORES):
        b = core // 4
        start = (core % 4) * S
        # x^T with halo, zero-padded at the left for chunk 0
        xt = np.zeros((EMBED, T), dtype=np.float32)
        lo = start - WINDOW
        src = x[b, max(lo, 0) : start + S, :]  # [<=640, e]
        xt[:, T - src.shape[0] :] = src.T
        xt = np.ascontiguousarray(xt.reshape(EMBED // P, P, T).transpose(1, 0, 2))
        # rope tables: query positions start..start+512, key positions lo..start+512
        qpos = np.arange(start, start + S, dtype=np.float64)
        kpos = np.maximum(np.arange(lo, start + S, dtype=np.float64), 0.0)
        scale = 1.0 / math.sqrt(HD)
        cq, sq = _rope_tables(qpos, scale)
        ck, sk = _rope_tables(kpos, 1.0)
        in_maps.append(
            {
                "XT": xt,
                "WQ": WQ,
                "WK": WK,
                "WV": WV,
                "WO": WOa,
                "COSQ": cq,
                "SINQ": sq,
                "COSK": ck,
                "SINK": sk,
                "MASKS": _band_masks(start),
                "ONES": np.ones((P, P), dtype=np.float32),
            }
        )
    return in_maps


def kernel(x, W_qkv, W_out, b_out):
    in_maps = prepare_in_maps(x, W_qkv, W_out, b_out)
    b_out = np.asarray(b_out, dtype=np.float32)

    if "nc" not in _CACHED:
        _CACHED["nc"] = build_bass()
    nc = _CACHED["nc"]

    res = run_bass_kernel_spmd(nc, in_maps, core_ids=list(range(NCORES)))
    LAST_RESULT["res"] = res

    out = np.empty((B, L, EMBED), dtype=np.float32)
    for core in range(NCORES):
        b = core // 4
        start = (core % 4) * S
        out[b, start : start + S, :] = res.results[core]["OUT"] + b_out
    return out


# revision 16
# speedup vs baseline: 1.0266x; 1.0266x over previous
"""Sliding-window (banded) multi-head self-attention on 8 trn2 NeuronCores.

Sequence-parallel sharding: batch b, 2048 tokens -> 4 chunks of 512 queries;
core c handles batch c//4, chunk c%4.  Each core receives x^T for its 512
tokens plus a 128-token halo (zero-padded for chunk 0), computes
qkv projection + RoPE + banded attention (window 129) + out projection for
its rows, and returns [512, 2048].  No cross-core communication.

Layout choices (all matmuls contract over the partition dim):
  - x^T resident in SBUF as [128, 16(e-chunk), 640(tok)]
  - Q^T/K^T per head feature-major [128(d), tok] straight out of PSUM;
    RoPE pairs de-interleaved host-side (d' = evens then odds) so
    rotate_half is a partition-half swap (SBUF->SBUF DMA).
  - V token-major [128(tok), d] (natural for PV lhsT).
  - scores^T computed per k-chunk as [128(k), 256(q)] fp32r matmuls;
    exp on ACT; 0/1 band-mask multiply; PV + replicated-ones rowsum
    matmuls accumulate into one PSUM bank; normalize with reciprocal.
  - out projection accumulates 16 hd-chunks; bias added host-side.

Scheduling notes (v2):
  - All SBUF<->SBUF swap DMAs and OUT stores are issued from the ACT
    engine's HWDGE queue so the SP queue never blocks on data-dependent
    waits and weight prefetch streams continuously.
  - Startup: xt chunk DMAs are interleaved with WQ/WK of head 0 so the
    PE starts accumulating Q0 while x is still streaming in; group 0's
    V projection is emitted ec-outer (5 parallel PSUM accumulators
    borrowed from the then-idle attention pools) so it proceeds at DMA
    arrival pace instead of stalling on the full 4.2MB WV block.
"""

import math
import numpy as np
import ml_dtypes

BF = ml_dtypes.bfloat16

import concourse.bass as bass
import concourse.tile as tile
from concourse import mybir
from concourse.bass_utils import run_bass_kernel_spmd
from concourse.vector_clock import ScopedClock, VectorClock


def _legalize_single_wait(nc):
    """This walrus build accepts only ONE sync-wait per lowered command
    ("Too many sync wait commands").  Move all but the last wait of every
    instruction onto single-wait NoOps prepended on the same engine: engines
    are in-order, so stalling on the NoOps is equivalent.  SP-issued DMAs are
    gated the same way (descriptor push happens in SP program order)."""
    nid = [0]
    for f in nc.m.functions:
        for blk in f.blocks:
            out = []
            changed = False
            for inst in blk.instructions:
                si = inst.sync_info
                waits = list(si.on_wait) if si and si.on_wait else []
                if len(waits) > 1:
                    changed = True
                    for w in waits[:-1]:
                        nop = mybir.InstNoOp(name=f"waitnop-{nid[0]}", ins=[], outs=[])
                        nid[0] += 1
                        nop.engine = inst.engine
                        nop.sync_info = mybir.SyncInfo(on_wait=[w], on_update=[])
                        out.append(nop)
                    inst.sync_info = mybir.SyncInfo(
                        on_wait=[waits[-1]], on_update=list(si.on_update or [])
                    )
                out.append(inst)
            if changed:
                blk.instructions = out
    return nc


def _install_drain_split_patch():
    """Split TileContext's closing drain into single-wait drains: walrus's
    CTRL_NO command rejects the catch-all drain ("Too many sync waits")."""
    if getattr(tile.TileContext, "_drain_split_patched", False):
        return

    def _patched(self, tick_clock, wait_clock):
        gvc = tick_clock.global_clock  # VectorClock over the 27 procs
        n = len(gvc)
        procs = [i for i in range(n) if gvc[i] > 0]
        for pi in procs:
            vc = VectorClock([gvc[i] if i == pi else 0 for i in range(n)])
            d = self.nc.sync.drain()
            wait_clock.add_sem_waits(d.ins, ScopedClock({None: vc}))
        self.nc.all_engine_barrier()
        assert self.sems is not None
        popped = self.nc._tile_sem_poison_stack.pop()
        assert popped is self._sem_poison
        self.nc.clear_and_free_semaphores(list(self.sems.allocated().values()))
        self.nc.all_engine_barrier()

    tile.TileContext._drain_and_barrier = _patched
    tile.TileContext._drain_split_patched = True


_install_drain_split_patch()

EMBED = 2048
HEADS = 16
HD = 128
WINDOW = 128
THETA = 10000.0
B = 2
L = 2048
S = 512            # queries per core
T = S + WINDOW     # k/v tokens per core (incl halo)
NCORES = 8
P = 128
F32 = mybir.dt.float32
F32R = mybir.dt.float32r
BF16 = mybir.dt.bfloat16


def build_bass(legalize=True):
    nc = bass.Bass("TRN2", target_bir_lowering=False, debug=False)

    XT = nc.dram_tensor("XT", [P, EMBED // P, T], F32R, kind="ExternalInput")
    WQ = nc.dram_tensor("WQ", [HEADS, P, EMBED // P, HD], F32R, kind="ExternalInput")
    WK = nc.dram_tensor("WK", [HEADS, P, EMBED // P, HD], F32R, kind="ExternalInput")
    WV = nc.dram_tensor("WV", [4, P, EMBED // P, 512], F32R, kind="ExternalInput")
    WO = nc.dram_tensor("WO", [4, P, EMBED // P, 512], F32R, kind="ExternalInput")
    COSQ = nc.dram_tensor("COSQ", [P, S], F32R, kind="ExternalInput")
    SINQ = nc.dram_tensor("SINQ", [P, S], BF16, kind="ExternalInput")
    COSK = nc.dram_tensor("COSK", [P, T], F32R, kind="ExternalInput")
    SINK = nc.dram_tensor("SINK", [P, T], BF16, kind="ExternalInput")
    MASKS = nc.dram_tensor("MASKS", [4, P, 256], BF16, kind="ExternalInput")
    ONES = nc.dram_tensor("ONES", [P, P], BF16, kind="ExternalInput")
    OUT = nc.dram_tensor("OUT", [S, EMBED], F32, kind="ExternalOutput")

    EC = EMBED // P  # 16 e-chunks
    GROUPS = 4       # head groups of 4 (for V projection at N=512)
    GH = HEADS // GROUPS
    HT = T // 2      # 320

    with tile.TileContext(nc) as tc:
        with (
            tc.tile_pool(name="persist", bufs=1) as persist,
            tc.tile_pool(name="wbig", bufs=8) as wbig,
            tc.tile_pool(name="rope_qk", bufs=4) as rope_qk,
            tc.tile_pool(name="rope_rs", bufs=6) as rope_rs,
            tc.tile_pool(name="vsb", bufs=9) as vsb_pool,
            tc.tile_pool(name="attn", bufs=4) as attn_pool,
            tc.tile_pool(name="small", bufs=2) as small,
            tc.tile_pool(name="outsb", bufs=2) as outsb,
            tc.tile_pool(name="ps_qv", bufs=2, space="PSUM") as ps_qv,
            tc.tile_pool(name="ps_k", bufs=1, space="PSUM") as ps_k,
            tc.tile_pool(name="ps_sc", bufs=2, space="PSUM") as ps_sc,
            tc.tile_pool(name="ps_oc", bufs=2, space="PSUM") as ps_oc,
        ):
            # ---- persistent tiles ----
            xt = persist.tile([P, EC, T], F32R, tag="xt")
            cosq = persist.tile([P, S], F32R, tag="cosq")
            sinq = persist.tile([P, S], BF16, tag="sinq")
            cosk = persist.tile([P, T], F32R, tag="cosk")
            sink = persist.tile([P, T], BF16, tag="sink")
            masks = persist.tile([P, 4, 256], BF16, tag="masks")
            ones_full = persist.tile([P, P], BF16, tag="ones_full")
            out_norm = persist.tile([P, HEADS, S], F32R, tag="out_norm")

            def load_wq(h, name):
                w = wbig.tile([P, EC, HD], F32R, tag="wbig", name=name)
                for i8 in range(2):
                    nc.sync.dma_start(
                        w[:, 8 * i8 : 8 * i8 + 8, :],
                        (WQ if name[1] == "q" else WK).ap()[
                            h, :, 8 * i8 : 8 * i8 + 8, :
                        ],
                    )
                return w

            def load_wv(g):
                wv_qs = []
                for q in range(4):
                    wvq = wbig.tile([P, 4, 512], F32R, tag="wbig", name=f"wv{g}_{q}")
                    nc.sync.dma_start(
                        wvq[:, 0:2, :], WV.ap()[g, :, 4 * q : 4 * q + 2, :]
                    )
                    nc.sync.dma_start(
                        wvq[:, 2:4, :], WV.ap()[g, :, 4 * q + 2 : 4 * q + 4, :]
                    )
                    wv_qs.append(wvq)
                return wv_qs

            # ---- startup DMA order: xt interleaved with wq0/wk0 ----
            wq0_sb = wbig.tile([P, EC, HD], F32R, tag="wbig", name="wq0")
            wk0_sb = wbig.tile([P, EC, HD], F32R, tag="wbig", name="wk0")
            for i4 in range(4):
                for ec in range(4 * i4, 4 * i4 + 4):
                    nc.sync.dma_start(xt[:, ec, :], XT.ap()[:, ec, :])
                nc.sync.dma_start(
                    wq0_sb[:, 4 * i4 : 4 * i4 + 4, :],
                    WQ.ap()[0, :, 4 * i4 : 4 * i4 + 4, :],
                )
                nc.sync.dma_start(
                    wk0_sb[:, 4 * i4 : 4 * i4 + 4, :],
                    WK.ap()[0, :, 4 * i4 : 4 * i4 + 4, :],
                )
            nc.scalar.dma_start(cosq, COSQ.ap())
            nc.scalar.dma_start(sinq, SINQ.ap())
            nc.scalar.dma_start(cosk, COSK.ap())
            nc.scalar.dma_start(sink, SINK.ap())

            def emit_qk_compute(h, wq_sb, wk_sb):
                # Q projection (feature-major [d, q]) + RoPE
                psq = ps_qv.tile([P, S], F32, tag="psqv", name=f"psq{h}")
                for ec in range(EC):
                    nc.tensor.matmul(
                        psq,
                        wq_sb[:, ec, :],
                        xt[:, ec, WINDOW:T],
                        start=(ec == 0),
                        stop=(ec == EC - 1),
                    )
                q_sb = rope.tile([P, S], F32R, tag="qrope", name=f"q{h}")
                qraw_t = rope.tile([P, T], F32R, tag="raw", name=f"qraw{h}")
                qraw = qraw_t[:, 0:S]
                nc.scalar.copy(qraw, psq)
                nc.vector.tensor_mul(q_sb, psq, cosq)
                qsw_t = rope.tile([P, T], F32R, tag="sw", name=f"qsw{h}")
                qsw = qsw_t[:, 0:S]
                nc.scalar.dma_start(qsw[0:64, :], qraw[64:128, :])
                nc.scalar.dma_start(qsw[64:128, :], qraw[0:64, :])
                nc.gpsimd.tensor_mul(qsw, qsw, sinq)
                nc.vector.tensor_add(q_sb, q_sb, qsw)

                # K projection + RoPE (tokens 0..640 in two 320 halves)
                psk1 = ps_k.tile([P, HT], F32, tag="psk1", name=f"psk1_{h}")
                psk2 = ps_k.tile([P, HT], F32, tag="psk2", name=f"psk2_{h}")
                for ec in range(EC):
                    nc.tensor.matmul(
                        psk1,
                        wk_sb[:, ec, :],
                        xt[:, ec, 0:HT],
                        start=(ec == 0),
                        stop=(ec == EC - 1),
                    )
                for ec in range(EC):
                    nc.tensor.matmul(
                        psk2,
                        wk_sb[:, ec, :],
                        xt[:, ec, HT:T],
                        start=(ec == 0),
                        stop=(ec == EC - 1),
                    )
                k_sb = rope.tile([P, T], F32R, tag="krope", name=f"k{h}")
                kraw = rope.tile([P, T], F32R, tag="raw", name=f"kraw{h}")
                nc.scalar.copy(kraw[:, 0:HT], psk1)
                nc.scalar.copy(kraw[:, HT:T], psk2)
                nc.vector.tensor_mul(k_sb[:, 0:HT], psk1, cosk[:, 0:HT])
                nc.vector.tensor_mul(k_sb[:, HT:T], psk2, cosk[:, HT:T])
                ksw = rope.tile([P, T], F32R, tag="sw", name=f"ksw{h}")
                nc.scalar.dma_start(ksw[0:64, :], kraw[64:128, :])
                nc.scalar.dma_start(ksw[64:128, :], kraw[0:64, :])
                nc.gpsimd.tensor_mul(ksw, ksw, sink)
                nc.vector.tensor_add(k_sb, k_sb, ksw)
                return q_sb, k_sb

            def emit_proj(h):
                wq_sb = load_wq(h, f"wq{h}")
                wk_sb = load_wq(h, f"wk{h}")
                return emit_qk_compute(h, wq_sb, wk_sb)

            def emit_vproj_steady(g, wv_qs):
                # weights already prefetched a group ahead: tt-outer is fine
                v_tiles = []
                for tt in range(T // P):  # 5 token tiles
                    psv = ps_qv.tile([P, 512], F32, tag="psqv", name=f"psv{g}_{tt}")
                    for ec in range(EC):
                        nc.tensor.matmul(
                            psv,
                            xt[:, ec, tt * P : (tt + 1) * P],
                            wv_qs[ec // 4][:, ec % 4, :],
                            start=(ec == 0),
                            stop=(ec == EC - 1),
                        )
                    v_sb = vsb_pool.tile([P, 512], BF16, tag="vsb", name=f"v{g}_{tt}")
                    nc.scalar.copy(v_sb, psv)
                    v_tiles.append(v_sb)
                return v_tiles

            def emit_vproj_g0(wv_qs):
                # ec-outer: 5 parallel accumulators so V proceeds at DMA pace.
                # Borrow banks from the (still idle) attention pools.
                psv = [
                    ps_oc.tile([P, 512], F32, tag="oc", name="psv0_0"),
                    ps_oc.tile([P, 512], F32, tag="oc", name="psv0_1"),
                    ps_sc.tile([P, 512], F32, tag="sc", name="psv0_2"),
                    ps_sc.tile([P, 512], F32, tag="sc", name="psv0_3"),
                    ps_qv.tile([P, 512], F32, tag="psqv", name="psv0_4"),
                ]
                for ec in range(EC):
                    for tt in range(T // P):
                        nc.tensor.matmul(
                            psv[tt],
                            xt[:, ec, tt * P : (tt + 1) * P],
                            wv_qs[ec // 4][:, ec % 4, :],
                            start=(ec == 0),
                            stop=(ec == EC - 1),
                        )
                v_tiles = []
                for tt in range(T // P):
                    v_sb = vsb_pool.tile([P, 512], BF16, tag="vsb", name=f"v0_{tt}")
                    nc.scalar.copy(v_sb, psv[tt])
                    v_tiles.append(v_sb)
                return v_tiles

            def emit_attn(h, q_sb, k_sb, v_tiles):
                hh = h % GH
                for p in range(2):
                    qs = p * 256
                    ets = []
                    for j in range(3):  # roles R1,R2,R3 -> k-chunk 2p+j
                        c = 2 * p + j
                        midx = 3 if (j == 0 and p == 1) else j
                        psc = ps_sc.tile([P, 256], F32, tag="sc", name=f"sc{h}_{p}{j}")
                        nc.tensor.matmul(
                            psc,
                            k_sb[:, c * P : (c + 1) * P],
                            q_sb[:, qs : qs + 256],
                            start=True,
                            stop=True,
                        )
                        et = attn_pool.tile([P, 256], BF16, tag="attn", name=f"et{h}_{p}{j}")
                        nc.scalar.activation(
                            et, psc, mybir.ActivationFunctionType.Exp
                        )
                        nc.vector.tensor_mul(et, et, masks[:, midx, :])
                        ets.append(et)

                    poc = ps_oc.tile([P, 512], F32, tag="oc", name=f"poc{h}_{p}")
                    for j in range(3):
                        c = 2 * p + j
                        nc.tensor.matmul(
                            poc[:, 0:256],
                            v_tiles[c][:, hh * HD : (hh + 1) * HD],
                            ets[j],
                            start=(j == 0),
                            stop=False,
                        )
                        nc.tensor.matmul(
                            poc[:, 256:512],
                            ones_full,
                            ets[j],
                            start=False,
                            stop=(j == 2),
                        )
                    recip = small.tile([P, 256], F32, tag="recip", name=f"rc{h}_{p}")
                    nc.vector.reciprocal(recip, poc[:, 256:512])
                    nc.vector.tensor_mul(
                        out_norm[:, h, qs : qs + 256], poc[:, 0:256], recip
                    )

            def load_wo(eo):
                wo_qs = []
                for q in range(4):
                    woq = wbig.tile([P, 4, 512], F32R, tag="wbig", name=f"wo{eo}_{q}")
                    nc.sync.dma_start(
                        woq[:, 0:2, :], WO.ap()[eo, :, 4 * q : 4 * q + 2, :]
                    )
                    nc.sync.dma_start(
                        woq[:, 2:4, :], WO.ap()[eo, :, 4 * q + 2 : 4 * q + 4, :]
                    )
                    wo_qs.append(woq)
                return wo_qs

            # ---- weight-issue lookahead: head h's swaps are emitted AFTER
            # the weight loads for heads h+1/h+2, so the SP queue parking on
            # a data-dependent swap never delays already-issued prefetch. ----
            wv_g0 = load_wv(0)
            nc.scalar.dma_start(masks, MASKS.ap().rearrange("m p q -> p m q"))
            nc.scalar.dma_start(ones_full, ONES.ap())
            wtiles = {0: (wq0_sb, wk0_sb)}
            wtiles[1] = (load_wq(1, "wq1"), load_wq(1, "wk1"))
            wv_pending = {0: wv_g0}
            pending = None
            wo_qs = None
            for h in range(HEADS):
                # issue weights two heads ahead
                ahead = h + 2
                if ahead < HEADS:
                    if ahead % GH == 0:
                        wv_pending[ahead // GH] = load_wv(ahead // GH)
                    wtiles[ahead] = (
                        load_wq(ahead, f"wq{ahead}"),
                        load_wq(ahead, f"wk{ahead}"),
                    )
                elif ahead == HEADS:
                    wo_qs = load_wo(0)
                qk = emit_qk_compute(h, *wtiles.pop(h))
                if pending is not None:
                    ph = pending[0]
                    emit_attn(ph, pending[1], pending[2], v_groups[ph // GH])
                pending = (h, qk[0], qk[1])
                if h % GH == 0:
                    g = h // GH
                    if g == 0:
                        v_groups = {0: emit_vproj_g0(wv_pending.pop(0))}
                    else:
                        v_groups[g] = emit_vproj_steady(g, wv_pending.pop(g))

            ph = pending[0]
            emit_attn(ph, pending[1], pending[2], v_groups[ph // GH])

            # ---- out projection: OUT[t, e] = sum_hd out_norm^T . WO ----
            wo_tiles = {0: wo_qs}
            for eo in range(4):
                e0 = eo * 512
                if eo + 1 < 4:
                    wo_tiles[eo + 1] = load_wo(eo + 1)
                wo_qs = wo_tiles.pop(eo)
                for tt in range(4):
                    pso = ps_oc.tile([P, 512], F32, tag="oc")
                    for hd in range(HEADS):
                        nc.tensor.matmul(
                            pso,
                            out_norm[:, hd, tt * P : (tt + 1) * P],
                            wo_qs[hd // 4][:, hd % 4, :],
                            start=(hd == 0),
                            stop=(hd == HEADS - 1),
                        )
                    o_sb = outsb.tile([P, 512], F32, tag="osb")
                    nc.scalar.copy(o_sb, pso)
                    nc.sync.dma_start(
                        OUT.ap()[tt * P : (tt + 1) * P, e0 : e0 + 512], o_sb
                    )

    if legalize:
        _legalize_single_wait(nc)
    return nc


def _rope_tables(pos, scale):
    """Feature-major [128, len(pos)] cos / sin' tables in de-interleaved d order.

    cos'[i, t] = cos(pos_t * invf[i % 64]) ; sin'[0:64] = -sin, sin'[64:128] = +sin.
    """
    inv_freq = 1.0 / (THETA ** (np.arange(0, HD, 2, dtype=np.float64) / HD))  # [64]
    ang = pos[None, :] * inv_freq[:, None]  # [64, T]
    cos = np.cos(ang)
    sin = np.sin(ang)
    cos_t = np.concatenate([cos, cos], axis=0) * scale
    sin_t = np.concatenate([-sin, sin], axis=0) * scale
    return cos_t.astype(np.float32), sin_t.astype(np.float32)


def _band_masks(start):
    """[4, 128, 256] multiplicative masks.

    Element (m, kp, qf): role m in {R1 pair0, R2, R3, R1 pair1};
    local key j = c*128 + kp, local query r = qs + qf;
    valid iff r <= j <= r + 128 and (global key) start - 128 + j >= 0.
    """
    out = np.zeros((4, P, 256), dtype=np.float32)
    roles = [(0, 0), (1, 0), (2, 0), (2, 256)]  # (chunk c, query offset qs)
    for m, (c, qs) in enumerate(roles):
        kp = np.arange(P)[:, None]
        qf = np.arange(256)[None, :]
        j = c * P + kp
        r = qs + qf
        valid = (r <= j) & (j <= r + WINDOW) & (start - WINDOW + j >= 0)
        out[m] = valid.astype(np.float32)
    return out


_CACHED = {}
LAST_RESULT = {}


def prepare_in_maps(x, W_qkv, W_out, b_out):
    x = np.asarray(x, dtype=np.float32)
    W_qkv = np.asarray(W_qkv, dtype=np.float32)
    W_out = np.asarray(W_out, dtype=np.float32)

    # host-side weight layout prep
    perm = np.concatenate([np.arange(0, HD, 2), np.arange(1, HD, 2)])  # de-interleave
    w4 = W_qkv.reshape(EMBED, HEADS, HD, 3)
    # [h, e, d] -> [h, p, ec, d] partition-major contiguous
    WQ = w4[..., 0].transpose(1, 0, 2)[:, :, perm].reshape(HEADS, EMBED // P, P, HD)
    WQ = np.ascontiguousarray(WQ.transpose(0, 2, 1, 3))
    WK = w4[..., 1].transpose(1, 0, 2)[:, :, perm].reshape(HEADS, EMBED // P, P, HD)
    WK = np.ascontiguousarray(WK.transpose(0, 2, 1, 3))
    # [e, f] -> [g, p, ec, 512]
    WV = w4[..., 2].reshape(EMBED // P, P, 4, 512)
    WV = np.ascontiguousarray(WV.transpose(2, 1, 0, 3))
    WOa = W_out.reshape(EMBED // P, P, 4, 512)
    WOa = np.ascontiguousarray(WOa.transpose(2, 1, 0, 3))

    in_maps = []
    for core in range(NCORES):
        b = core // 4
        start = (core % 4) * S
        # x^T with halo, zero-padded at the left for chunk 0
        xt = np.zeros((EMBED, T), dtype=np.float32)
        lo = start - WINDOW
        src = x[b, max(lo, 0) : start + S, :]  # [<=640, e]
        xt[:, T - src.shape[0] :] = src.T
        xt = np.ascontiguousarray(xt.reshape(EMBED // P, P, T).transpose(1, 0, 2))
        # rope tables: query positions start..start+512, key positions lo..start+512
        qpos = np.arange(start, start + S, dtype=np.float64)
        kpos = np.maximum(np.arange(lo, start + S, dtype=np.float64), 0.0)
        scale = 1.0 / math.sqrt(HD)
        cq, sq = _rope_tables(qpos, scale)
        ck, sk = _rope_tables(kpos, 1.0)
        in_maps.append(
            {
                "XT": xt,
                "WQ": WQ,
                "WK": WK,
                "WV": WV,
                "WO": WOa,
                "COSQ": cq,
                "SINQ": sq.astype(BF),
                "COSK": ck,
                "SINK": sk.astype(BF),
                "MASKS": _band_masks(start).astype(BF),
                "ONES": np.ones((P, P), dtype=BF),
            }
        )
    return in_maps


def kernel(x, W_qkv, W_out, b_out):
    in_maps = prepare_in_maps(x, W_qkv, W_out, b_out)
    b_out = np.asarray(b_out, dtype=np.float32)

    if "nc" not in _CACHED:
        _CACHED["nc"] = build_bass()
    nc = _CACHED["nc"]

    res = run_bass_kernel_spmd(nc, in_maps, core_ids=list(range(NCORES)))
    LAST_RESULT["res"] = res

    out = np.empty((B, L, EMBED), dtype=np.float32)
    for core in range(NCORES):
        b = core // 4
        start = (core % 4) * S
        out[b, start : start + S, :] = res.results[core]["OUT"] + b_out
    return out


# revision 17
# speedup vs baseline: 1.2080x; 1.1767x over previous
"""Sliding-window (banded) multi-head self-attention on 8 trn2 NeuronCores.

Sequence-parallel sharding: batch b, 2048 tokens -> 4 chunks of 512 queries;
core c handles batch c//4, chunk c%4.  Each core receives x^T for its 512
tokens plus a 128-token halo (zero-padded for chunk 0), computes
qkv projection + RoPE + banded attention (window 129) + out projection for
its rows, and returns [512, 2048].  No cross-core communication.

Layout choices (all matmuls contract over the partition dim):
  - x^T resident in SBUF as [128, 16(e-chunk), 640(tok)]
  - Q^T/K^T per head feature-major [128(d), tok] straight out of PSUM;
    RoPE pairs de-interleaved host-side (d' = evens then odds) so
    rotate_half is a partition-half swap (SBUF->SBUF DMA).
  - V token-major [128(tok), d] (natural for PV lhsT).
  - scores^T computed per k-chunk as [128(k), 256(q)] fp32r matmuls;
    exp on ACT; 0/1 band-mask multiply; PV + replicated-ones rowsum
    matmuls accumulate into one PSUM bank; normalize with reciprocal.
  - out projection accumulates 16 hd-chunks; bias added host-side.

Scheduling notes (v2):
  - All SBUF<->SBUF swap DMAs and OUT stores are issued from the ACT
    engine's HWDGE queue so the SP queue never blocks on data-dependent
    waits and weight prefetch streams continuously.
  - Startup: xt chunk DMAs are interleaved with WQ/WK of head 0 so the
    PE starts accumulating Q0 while x is still streaming in; group 0's
    V projection is emitted ec-outer (5 parallel PSUM accumulators
    borrowed from the then-idle attention pools) so it proceeds at DMA
    arrival pace instead of stalling on the full 4.2MB WV block.
"""

import math
import numpy as np
import ml_dtypes

BF = ml_dtypes.bfloat16

import concourse.bass as bass
import concourse.tile as tile
from concourse import mybir
from concourse.bass_utils import run_bass_kernel_spmd
from concourse.vector_clock import ScopedClock, VectorClock


def _legalize_single_wait(nc):
    """This walrus build accepts only ONE sync-wait per lowered command
    ("Too many sync wait commands").  Move all but the last wait of every
    instruction onto single-wait NoOps prepended on the same engine: engines
    are in-order, so stalling on the NoOps is equivalent.  SP-issued DMAs are
    gated the same way (descriptor push happens in SP program order)."""
    nid = [0]
    for f in nc.m.functions:
        for blk in f.blocks:
            out = []
            changed = False
            for inst in blk.instructions:
                si = inst.sync_info
                waits = list(si.on_wait) if si and si.on_wait else []
                if len(waits) > 1:
                    changed = True
                    for w in waits[:-1]:
                        nop = mybir.InstNoOp(name=f"waitnop-{nid[0]}", ins=[], outs=[])
                        nid[0] += 1
                        nop.engine = inst.engine
                        nop.sync_info = mybir.SyncInfo(on_wait=[w], on_update=[])
                        out.append(nop)
                    inst.sync_info = mybir.SyncInfo(
                        on_wait=[waits[-1]], on_update=list(si.on_update or [])
                    )
                out.append(inst)
            if changed:
                blk.instructions = out
    return nc


def _install_drain_split_patch():
    """Split TileContext's closing drain into single-wait drains: walrus's
    CTRL_NO command rejects the catch-all drain ("Too many sync waits")."""
    if getattr(tile.TileContext, "_drain_split_patched", False):
        return

    def _patched(self, tick_clock, wait_clock):
        gvc = tick_clock.global_clock  # VectorClock over the 27 procs
        n = len(gvc)
        procs = [i for i in range(n) if gvc[i] > 0]
        for pi in procs:
            vc = VectorClock([gvc[i] if i == pi else 0 for i in range(n)])
            d = self.nc.sync.drain()
            wait_clock.add_sem_waits(d.ins, ScopedClock({None: vc}))
        self.nc.all_engine_barrier()
        assert self.sems is not None
        popped = self.nc._tile_sem_poison_stack.pop()
        assert popped is self._sem_poison
        self.nc.clear_and_free_semaphores(list(self.sems.allocated().values()))
        self.nc.all_engine_barrier()

    tile.TileContext._drain_and_barrier = _patched
    tile.TileContext._drain_split_patched = True


_install_drain_split_patch()

EMBED = 2048
HEADS = 16
HD = 128
WINDOW = 128
THETA = 10000.0
B = 2
L = 2048
S = 512            # queries per core
T = S + WINDOW     # k/v tokens per core (incl halo)
NCORES = 8
P = 128
F32 = mybir.dt.float32
F32R = mybir.dt.float32r
BF16 = mybir.dt.bfloat16


def build_bass(legalize=True):
    nc = bass.Bass("TRN2", target_bir_lowering=False, debug=False)

    XT = nc.dram_tensor("XT", [P, EMBED // P, T], F32R, kind="ExternalInput")
    WQ = nc.dram_tensor("WQ", [HEADS, P, EMBED // P, HD], F32R, kind="ExternalInput")
    WK = nc.dram_tensor("WK", [HEADS, P, EMBED // P, HD], F32R, kind="ExternalInput")
    WV = nc.dram_tensor("WV", [4, P, EMBED // P, 512], F32R, kind="ExternalInput")
    WO = nc.dram_tensor("WO", [4, P, EMBED // P, 512], F32R, kind="ExternalInput")
    COSQ = nc.dram_tensor("COSQ", [P, S], F32R, kind="ExternalInput")
    SINQ = nc.dram_tensor("SINQ", [P, S], BF16, kind="ExternalInput")
    COSK = nc.dram_tensor("COSK", [P, T], F32R, kind="ExternalInput")
    SINK = nc.dram_tensor("SINK", [P, T], BF16, kind="ExternalInput")
    MASKS = nc.dram_tensor("MASKS", [4, P, 256], BF16, kind="ExternalInput")
    ONES = nc.dram_tensor("ONES", [P, P], BF16, kind="ExternalInput")
    OUT = nc.dram_tensor("OUT", [S, EMBED], F32, kind="ExternalOutput")

    EC = EMBED // P  # 16 e-chunks
    GROUPS = 4       # head groups of 4 (for V projection at N=512)
    GH = HEADS // GROUPS
    HT = T // 2      # 320

    with tile.TileContext(nc) as tc:
        with (
            tc.tile_pool(name="persist", bufs=1) as persist,
            tc.tile_pool(name="wbig", bufs=8) as wbig,
            tc.tile_pool(name="rope_qk", bufs=4) as rope_qk,
            tc.tile_pool(name="rope_rs", bufs=6) as rope_rs,
            tc.tile_pool(name="vsb", bufs=9) as vsb_pool,
            tc.tile_pool(name="attn", bufs=4) as attn_pool,
            tc.tile_pool(name="small", bufs=2) as small,
            tc.tile_pool(name="outsb", bufs=2) as outsb,
            tc.tile_pool(name="ps_qv", bufs=2, space="PSUM") as ps_qv,
            tc.tile_pool(name="ps_k", bufs=1, space="PSUM") as ps_k,
            tc.tile_pool(name="ps_sc", bufs=2, space="PSUM") as ps_sc,
            tc.tile_pool(name="ps_oc", bufs=2, space="PSUM") as ps_oc,
        ):
            # ---- persistent tiles ----
            xt = persist.tile([P, EC, T], F32R, tag="xt")
            cosq = persist.tile([P, S], F32R, tag="cosq")
            sinq = persist.tile([P, S], BF16, tag="sinq")
            cosk = persist.tile([P, T], F32R, tag="cosk")
            sink = persist.tile([P, T], BF16, tag="sink")
            masks = persist.tile([P, 4, 256], BF16, tag="masks")
            ones_full = persist.tile([P, P], BF16, tag="ones_full")
            out_norm = persist.tile([P, HEADS, S], F32R, tag="out_norm")

            def load_wq(h, name):
                w = wbig.tile([P, EC, HD], F32R, tag="wbig", name=name)
                for i8 in range(2):
                    nc.sync.dma_start(
                        w[:, 8 * i8 : 8 * i8 + 8, :],
                        (WQ if name[1] == "q" else WK).ap()[
                            h, :, 8 * i8 : 8 * i8 + 8, :
                        ],
                    )
                return w

            def load_wv(g):
                wv_qs = []
                for q in range(4):
                    wvq = wbig.tile([P, 4, 512], F32R, tag="wbig", name=f"wv{g}_{q}")
                    nc.sync.dma_start(
                        wvq[:, 0:2, :], WV.ap()[g, :, 4 * q : 4 * q + 2, :]
                    )
                    nc.sync.dma_start(
                        wvq[:, 2:4, :], WV.ap()[g, :, 4 * q + 2 : 4 * q + 4, :]
                    )
                    wv_qs.append(wvq)
                return wv_qs

            # ---- startup DMA order: xt interleaved with wq0/wk0 ----
            wq0_sb = wbig.tile([P, EC, HD], F32R, tag="wbig", name="wq0")
            wk0_sb = wbig.tile([P, EC, HD], F32R, tag="wbig", name="wk0")
            for i4 in range(4):
                for ec in range(4 * i4, 4 * i4 + 4):
                    nc.sync.dma_start(xt[:, ec, :], XT.ap()[:, ec, :])
                nc.sync.dma_start(
                    wq0_sb[:, 4 * i4 : 4 * i4 + 4, :],
                    WQ.ap()[0, :, 4 * i4 : 4 * i4 + 4, :],
                )
                nc.sync.dma_start(
                    wk0_sb[:, 4 * i4 : 4 * i4 + 4, :],
                    WK.ap()[0, :, 4 * i4 : 4 * i4 + 4, :],
                )
            nc.scalar.dma_start(cosq, COSQ.ap())
            nc.scalar.dma_start(sinq, SINQ.ap())
            nc.scalar.dma_start(cosk, COSK.ap())
            nc.scalar.dma_start(sink, SINK.ap())

            def emit_qk_compute(h, wq_sb, wk_sb):
                # Q projection (feature-major [d, q]) + RoPE
                psq = ps_qv.tile([P, S], F32, tag="psqv", name=f"psq{h}")
                for ec in range(EC):
                    nc.tensor.matmul(
                        psq,
                        wq_sb[:, ec, :],
                        xt[:, ec, WINDOW:T],
                        start=(ec == 0),
                        stop=(ec == EC - 1),
                    )
                q_sb = rope.tile([P, S], F32R, tag="qrope", name=f"q{h}")
                qraw_t = rope.tile([P, T], F32R, tag="raw", name=f"qraw{h}")
                qraw = qraw_t[:, 0:S]
                nc.scalar.copy(qraw, psq)
                nc.vector.tensor_mul(q_sb, psq, cosq)
                qsw_t = rope.tile([P, T], F32R, tag="sw", name=f"qsw{h}")
                qsw = qsw_t[:, 0:S]
                nc.sync.dma_start(qsw[0:64, :], qraw[64:128, :])
                nc.sync.dma_start(qsw[64:128, :], qraw[0:64, :])
                nc.gpsimd.tensor_mul(qsw, qsw, sinq)
                nc.vector.tensor_add(q_sb, q_sb, qsw)

                # K projection + RoPE (tokens 0..640 in two 320 halves)
                psk1 = ps_k.tile([P, HT], F32, tag="psk1", name=f"psk1_{h}")
                psk2 = ps_k.tile([P, HT], F32, tag="psk2", name=f"psk2_{h}")
                for ec in range(EC):
                    nc.tensor.matmul(
                        psk1,
                        wk_sb[:, ec, :],
                        xt[:, ec, 0:HT],
                        start=(ec == 0),
                        stop=(ec == EC - 1),
                    )
                for ec in range(EC):
                    nc.tensor.matmul(
                        psk2,
                        wk_sb[:, ec, :],
                        xt[:, ec, HT:T],
                        start=(ec == 0),
                        stop=(ec == EC - 1),
                    )
                k_sb = rope.tile([P, T], F32R, tag="krope", name=f"k{h}")
                kraw = rope.tile([P, T], F32R, tag="raw", name=f"kraw{h}")
                nc.scalar.copy(kraw[:, 0:HT], psk1)
                nc.scalar.copy(kraw[:, HT:T], psk2)
                nc.vector.tensor_mul(k_sb[:, 0:HT], psk1, cosk[:, 0:HT])
                nc.vector.tensor_mul(k_sb[:, HT:T], psk2, cosk[:, HT:T])
                ksw = rope.tile([P, T], F32R, tag="sw", name=f"ksw{h}")
                nc.sync.dma_start(ksw[0:64, :], kraw[64:128, :])
                nc.sync.dma_start(ksw[64:128, :], kraw[0:64, :])
                nc.gpsimd.tensor_mul(ksw, ksw, sink)
                nc.vector.tensor_add(k_sb, k_sb, ksw)
                return q_sb, k_sb

            def emit_proj(h):
                wq_sb = load_wq(h, f"wq{h}")
                wk_sb = load_wq(h, f"wk{h}")
                return emit_qk_compute(h, wq_sb, wk_sb)

            def emit_vproj_steady(g, wv_qs):
                # weights already prefetched a group ahead: tt-outer is fine
                v_tiles = []
                for tt in range(T // P):  # 5 token tiles
                    psv = ps_qv.tile([P, 512], F32, tag="psqv", name=f"psv{g}_{tt}")
                    for ec in range(EC):
                        nc.tensor.matmul(
                            psv,
                            xt[:, ec, tt * P : (tt + 1) * P],
                            wv_qs[ec // 4][:, ec % 4, :],
                            start=(ec == 0),
                            stop=(ec == EC - 1),
                        )
                    v_sb = vsb_pool.tile([P, 512], BF16, tag="vsb", name=f"v{g}_{tt}")
                    nc.scalar.copy(v_sb, psv)
                    v_tiles.append(v_sb)
                return v_tiles

            def emit_vproj_g0(wv_qs):
                # ec-outer: 5 parallel accumulators so V proceeds at DMA pace.
                # Borrow banks from the (still idle) attention pools.
                psv = [
                    ps_oc.tile([P, 512], F32, tag="oc", name="psv0_0"),
                    ps_oc.tile([P, 512], F32, tag="oc", name="psv0_1"),
                    ps_sc.tile([P, 512], F32, tag="sc", name="psv0_2"),
                    ps_sc.tile([P, 512], F32, tag="sc", name="psv0_3"),
                    ps_qv.tile([P, 512], F32, tag="psqv", name="psv0_4"),
                ]
                for ec in range(EC):
                    for tt in range(T // P):
                        nc.tensor.matmul(
                            psv[tt],
                            xt[:, ec, tt * P : (tt + 1) * P],
                            wv_qs[ec // 4][:, ec % 4, :],
                            start=(ec == 0),
                            stop=(ec == EC - 1),
                        )
                v_tiles = []
                for tt in range(T // P):
                    v_sb = vsb_pool.tile([P, 512], BF16, tag="vsb", name=f"v0_{tt}")
                    nc.scalar.copy(v_sb, psv[tt])
                    v_tiles.append(v_sb)
                return v_tiles

            def emit_attn(h, q_sb, k_sb, v_tiles):
                hh = h % GH
                for p in range(2):
                    qs = p * 256
                    ets = []
                    for j in range(3):  # roles R1,R2,R3 -> k-chunk 2p+j
                        c = 2 * p + j
                        midx = 3 if (j == 0 and p == 1) else j
                        psc = ps_sc.tile([P, 256], F32, tag="sc", name=f"sc{h}_{p}{j}")
                        nc.tensor.matmul(
                            psc,
                            k_sb[:, c * P : (c + 1) * P],
                            q_sb[:, qs : qs + 256],
                            start=True,
                            stop=True,
                        )
                        et = attn_pool.tile([P, 256], BF16, tag="attn", name=f"et{h}_{p}{j}")
                        nc.scalar.activation(
                            et, psc, mybir.ActivationFunctionType.Exp
                        )
                        nc.vector.tensor_mul(et, et, masks[:, midx, :])
                        ets.append(et)

                    poc = ps_oc.tile([P, 512], F32, tag="oc", name=f"poc{h}_{p}")
                    for j in range(3):
                        c = 2 * p + j
                        nc.tensor.matmul(
                            poc[:, 0:256],
                            v_tiles[c][:, hh * HD : (hh + 1) * HD],
                            ets[j],
                            start=(j == 0),
                            stop=False,
                        )
                        nc.tensor.matmul(
                            poc[:, 256:512],
                            ones_full,
                            ets[j],
                            start=False,
                            stop=(j == 2),
                        )
                    recip = small.tile([P, 256], F32, tag="recip", name=f"rc{h}_{p}")
                    nc.vector.reciprocal(recip, poc[:, 256:512])
                    nc.vector.tensor_mul(
                        out_norm[:, h, qs : qs + 256], poc[:, 0:256], recip
                    )

            def load_wo(eo):
                wo_qs = []
                for q in range(4):
                    woq = wbig.tile([P, 4, 512], F32R, tag="wbig", name=f"wo{eo}_{q}")
                    nc.sync.dma_start(
                        woq[:, 0:2, :], WO.ap()[eo, :, 4 * q : 4 * q + 2, :]
                    )
                    nc.sync.dma_start(
                        woq[:, 2:4, :], WO.ap()[eo, :, 4 * q + 2 : 4 * q + 4, :]
                    )
                    wo_qs.append(woq)
                return wo_qs

            # ---- weight-issue lookahead: head h's swaps are emitted AFTER
            # the weight loads for heads h+1/h+2, so the SP queue parking on
            # a data-dependent swap never delays already-issued prefetch. ----
            wv_g0 = load_wv(0)
            nc.scalar.dma_start(masks, MASKS.ap().rearrange("m p q -> p m q"))
            nc.scalar.dma_start(ones_full, ONES.ap())
            wtiles = {0: (wq0_sb, wk0_sb)}
            wtiles[1] = (load_wq(1, "wq1"), load_wq(1, "wk1"))
            wv_pending = {0: wv_g0}
            pending = None
            wo_qs = None
            for h in range(HEADS):
                # issue weights two heads ahead
                ahead = h + 2
                if ahead < HEADS:
                    if ahead % GH == 0:
                        wv_pending[ahead // GH] = load_wv(ahead // GH)
                    wtiles[ahead] = (
                        load_wq(ahead, f"wq{ahead}"),
                        load_wq(ahead, f"wk{ahead}"),
                    )
                elif ahead == HEADS:
                    wo_qs = load_wo(0)
                qk = emit_qk_compute(h, *wtiles.pop(h))
                if pending is not None:
                    ph = pending[0]
                    emit_attn(ph, pending[1], pending[2], v_groups[ph // GH])
                pending = (h, qk[0], qk[1])
                if h % GH == 0:
                    g = h // GH
                    if g == 0:
                        v_groups = {0: emit_vproj_g0(wv_pending.pop(0))}
                    else:
                        v_groups[g] = emit_vproj_steady(g, wv_pending.pop(g))

            ph = pending[0]
            emit_attn(ph, pending[1], pending[2], v_groups[ph // GH])

            # ---- out projection: OUT[t, e] = sum_hd out_norm^T . WO ----
            wo_tiles = {0: wo_qs}
            for eo in range(4):
                e0 = eo * 512
                if eo + 1 < 4:
                    wo_tiles[eo + 1] = load_wo(eo + 1)
                wo_qs = wo_tiles.pop(eo)
                for tt in range(4):
                    pso = ps_oc.tile([P, 512], F32, tag="oc")
                    for hd in range(HEADS):
                        nc.tensor.matmul(
                            pso,
                            out_norm[:, hd, tt * P : (tt + 1) * P],
                            wo_qs[hd // 4][:, hd % 4, :],
                            start=(hd == 0),
                            stop=(hd == HEADS - 1),
                        )
                    o_sb = outsb.tile([P, 512], F32, tag="osb")
                    nc.scalar.copy(o_sb, pso)
                    nc.scalar.dma_start(
                        OUT.ap()[tt * P : (tt + 1) * P, e0 : e0 + 512], o_sb
                    )

    if legalize:
        _legalize_single_wait(nc)
    return nc


def _rope_tables(pos, scale):
    """Feature-major [128, len(pos)] cos / sin' tables in de-interleaved d order.

    cos'[i, t] = cos(pos_t * invf[i % 64]) ; sin'[0:64] = -sin, sin'[64:128] = +sin.
    """
    inv_freq = 1.0 / (THETA ** (np.arange(0, HD, 2, dtype=np.float64) / HD))  # [64]
    ang = pos[None, :] * inv_freq[:, None]  # [64, T]
    cos = np.cos(ang)
    sin = np.sin(ang)
    cos_t = np.concatenate([cos, cos], axis=0) * scale
    sin_t = np.concatenate([-sin, sin], axis=0) * scale
    return cos_t.astype(np.float32), sin_t.astype(np.float32)


def _band_masks(start):
    """[4, 128, 256] multiplicative masks.

    Element (m, kp, qf): role m in {R1 pair0, R2, R3, R1 pair1};
    local key j = c*128 + kp, local query r = qs + qf;
    valid iff r <= j <= r + 128 and (global key) start - 128 + j >= 0.
    """
    out = np.zeros((4, P, 256), dtype=np.float32)
    roles = [(0, 0), (1, 0), (2, 0), (2, 256)]  # (chunk c, query offset qs)
    for m, (c, qs) in enumerate(roles):
        kp = np.arange(P)[:, None]
        qf = np.arange(256)[None, :]
        j = c * P + kp
        r = qs + qf
        valid = (r <= j) & (j <= r + WINDOW) & (start - WINDOW + j >= 0)
        out[m] = valid.astype(np.float32)
    return out


_CACHED = {}
LAST_RESULT = {}


def prepare_in_maps(x, W_qkv, W_out, b_out):
    x = np.asarray(x, dtype=np.float32)
    W_qkv = np.asarray(W_qkv, dtype=np.float32)
    W_out = np.asarray(W_out, dtype=np.float32)

    # host-side weight layout prep
    perm = np.concatenate([np.arange(0, HD, 2), np.arange(1, HD, 2)])  # de-interleave
    w4 = W_qkv.reshape(EMBED, HEADS, HD, 3)
    # [h, e, d] -> [h, p, ec, d] partition-major contiguous
    WQ = w4[..., 0].transpose(1, 0, 2)[:, :, perm].reshape(HEADS, EMBED // P, P, HD)
    WQ = np.ascontiguousarray(WQ.transpose(0, 2, 1, 3))
    WK = w4[..., 1].transpose(1, 0, 2)[:, :, perm].reshape(HEADS, EMBED // P, P, HD)
    WK = np.ascontiguousarray(WK.transpose(0, 2, 1, 3))
    # [e, f] -> [g, p, ec, 512]
    WV = w4[..., 2].reshape(EMBED // P, P, 4, 512)
    WV = np.ascontiguousarray(WV.transpose(2, 1, 0, 3))
    WOa = W_out.reshape(EMBED // P, P, 4, 512)
    WOa = np.ascontiguousarray(WOa.transpose(2, 1, 0, 3))

    in_maps = []
    for core in range(NCORES):
        b = core // 4
        start = (core % 4) * S
        # x^T with halo, zero-padded at the left for chunk 0
        xt = np.zeros((EMBED, T), dtype=np.float32)
        lo = start - WINDOW
        src = x[b, max(lo, 0) : start + S, :]  # [<=640, e]
        xt[:, T - src.shape[0] :] = src.T
        xt = np.ascontiguousarray(xt.reshape(EMBED // P, P, T).transpose(1, 0, 2))
        # rope tables: query positions start..start+512, key positions lo..start+512
        qpos = np.arange(start, start + S, dtype=np.float64)
        kpos = np.maximum(np.arange(lo, start + S, dtype=np.float64), 0.0)
        scale = 1.0 / math.sqrt(HD)
        cq, sq = _rope_tables(qpos, scale)
        ck, sk = _rope_tables(kpos, 1.0)
        in_maps.append(
            {
                "XT": xt,
                "WQ": WQ,
                "WK": WK,
                "WV": WV,
                "WO": WOa,
                "COSQ": cq,
                "SINQ": sq.astype(BF),
                "COSK": ck,
                "SINK": sk.astype(BF),
                "MASKS": _band_masks(start).astype(BF),
                "ONES": np.ones((P, P), dtype=BF),
            }
        )
    return in_maps


def kernel(x, W_qkv, W_out, b_out):
    in_maps = prepare_in_maps(x, W_qkv, W_out, b_out)
    b_out = np.asarray(b_out, dtype=np.float32)

    if "nc" not in _CACHED:
        _CACHED["nc"] = build_bass()
    nc = _CACHED["nc"]

    res = run_bass_kernel_spmd(nc, in_maps, core_ids=list(range(NCORES)))
    LAST_RESULT["res"] = res

    out = np.empty((B, L, EMBED), dtype=np.float32)
    for core in range(NCORES):
        b = core // 4
        start = (core % 4) * S
        out[b, start : start + S, :] = res.results[core]["OUT"] + b_out
    return out


# revision 18
# speedup vs baseline: 1.2144x; 1.0053x over previous
"""Sliding-window (banded) multi-head self-attention on 8 trn2 NeuronCores.

Sequence-parallel sharding: batch b, 2048 tokens -> 4 chunks of 512 queries;
core c handles batch c//4, chunk c%4.  Each core receives x^T for its 512
tokens plus a 128-token halo (zero-padded for chunk 0), computes
qkv projection + RoPE + banded attention (window 129) + out projection for
its rows, and returns [512, 2048].  No cross-core communication.

Layout choices (all matmuls contract over the partition dim):
  - x^T resident in SBUF as [128, 16(e-chunk), 640(tok)]
  - Q^T/K^T per head feature-major [128(d), tok] straight out of PSUM;
    RoPE pairs de-interleaved host-side (d' = evens then odds) so
    rotate_half is a partition-half swap (SBUF->SBUF DMA).
  - V token-major [128(tok), d] (natural for PV lhsT).
  - scores^T computed per k-chunk as [128(k), 256(q)] fp32r matmuls;
    exp on ACT; 0/1 band-mask multiply; PV + replicated-ones rowsum
    matmuls accumulate into one PSUM bank; normalize with reciprocal.
  - out projection accumulates 16 hd-chunks; bias added host-side.

Scheduling notes (v2):
  - All SBUF<->SBUF swap DMAs and OUT stores are issued from the ACT
    engine's HWDGE queue so the SP queue never blocks on data-dependent
    waits and weight prefetch streams continuously.
  - Startup: xt chunk DMAs are interleaved with WQ/WK of head 0 so the
    PE starts accumulating Q0 while x is still streaming in; group 0's
    V projection is emitted ec-outer (5 parallel PSUM accumulators
    borrowed from the then-idle attention pools) so it proceeds at DMA
    arrival pace instead of stalling on the full 4.2MB WV block.
"""

import math
import numpy as np
import ml_dtypes

BF = ml_dtypes.bfloat16

import concourse.bass as bass
import concourse.tile as tile
from concourse import mybir
from concourse.bass_utils import run_bass_kernel_spmd
from concourse.vector_clock import ScopedClock, VectorClock


def _legalize_single_wait(nc):
    """This walrus build accepts only ONE sync-wait per lowered command
    ("Too many sync wait commands").  Move all but the last wait of every
    instruction onto single-wait NoOps prepended on the same engine: engines
    are in-order, so stalling on the NoOps is equivalent.  SP-issued DMAs are
    gated the same way (descriptor push happens in SP program order)."""
    nid = [0]
    for f in nc.m.functions:
        for blk in f.blocks:
            out = []
            changed = False
            for inst in blk.instructions:
                si = inst.sync_info
                waits = list(si.on_wait) if si and si.on_wait else []
                if len(waits) > 1:
                    changed = True
                    for w in waits[:-1]:
                        nop = mybir.InstNoOp(name=f"waitnop-{nid[0]}", ins=[], outs=[])
                        nid[0] += 1
                        nop.engine = inst.engine
                        nop.sync_info = mybir.SyncInfo(on_wait=[w], on_update=[])
                        out.append(nop)
                    inst.sync_info = mybir.SyncInfo(
                        on_wait=[waits[-1]], on_update=list(si.on_update or [])
                    )
                out.append(inst)
            if changed:
                blk.instructions = out
    return nc


def _install_drain_split_patch():
    """Split TileContext's closing drain into single-wait drains: walrus's
    CTRL_NO command rejects the catch-all drain ("Too many sync waits")."""
    if getattr(tile.TileContext, "_drain_split_patched", False):
        return

    def _patched(self, tick_clock, wait_clock):
        gvc = tick_clock.global_clock  # VectorClock over the 27 procs
        n = len(gvc)
        procs = [i for i in range(n) if gvc[i] > 0]
        for pi in procs:
            vc = VectorClock([gvc[i] if i == pi else 0 for i in range(n)])
            d = self.nc.sync.drain()
            wait_clock.add_sem_waits(d.ins, ScopedClock({None: vc}))
        self.nc.all_engine_barrier()
        assert self.sems is not None
        popped = self.nc._tile_sem_poison_stack.pop()
        assert popped is self._sem_poison
        self.nc.clear_and_free_semaphores(list(self.sems.allocated().values()))
        self.nc.all_engine_barrier()

    tile.TileContext._drain_and_barrier = _patched
    tile.TileContext._drain_split_patched = True


_install_drain_split_patch()

EMBED = 2048
HEADS = 16
HD = 128
WINDOW = 128
THETA = 10000.0
B = 2
L = 2048
S = 512            # queries per core
T = S + WINDOW     # k/v tokens per core (incl halo)
NCORES = 8
P = 128
F32 = mybir.dt.float32
F32R = mybir.dt.float32r
BF16 = mybir.dt.bfloat16


def build_bass(legalize=True):
    nc = bass.Bass("TRN2", target_bir_lowering=False, debug=False)

    XT = nc.dram_tensor("XT", [P, EMBED // P, T], F32R, kind="ExternalInput")
    WQ = nc.dram_tensor("WQ", [HEADS, P, EMBED // P, HD], F32R, kind="ExternalInput")
    WK = nc.dram_tensor("WK", [HEADS, P, EMBED // P, HD], F32R, kind="ExternalInput")
    WV = nc.dram_tensor("WV", [4, P, EMBED // P, 512], F32R, kind="ExternalInput")
    WO = nc.dram_tensor("WO", [4, P, EMBED // P, 512], F32R, kind="ExternalInput")
    COSQ = nc.dram_tensor("COSQ", [P, S], F32R, kind="ExternalInput")
    SINQ = nc.dram_tensor("SINQ", [P, S], BF16, kind="ExternalInput")
    COSK = nc.dram_tensor("COSK", [P, T], F32R, kind="ExternalInput")
    SINK = nc.dram_tensor("SINK", [P, T], BF16, kind="ExternalInput")
    MASKS = nc.dram_tensor("MASKS", [4, P, 256], BF16, kind="ExternalInput")
    ONES = nc.dram_tensor("ONES", [P, P], BF16, kind="ExternalInput")
    OUT = nc.dram_tensor("OUT", [S, EMBED], F32, kind="ExternalOutput")
    DBG = nc.dram_tensor("DBG", [1, 16], F32, kind="ExternalOutput")

    EC = EMBED // P  # 16 e-chunks
    GROUPS = 4       # head groups of 4 (for V projection at N=512)
    GH = HEADS // GROUPS
    HT = T // 2      # 320

    with tile.TileContext(nc) as tc:
        with (
            tc.tile_pool(name="persist", bufs=1) as persist,
            tc.tile_pool(name="wbig", bufs=8) as wbig,
            tc.tile_pool(name="rope_qk", bufs=4) as rope_qk,
            tc.tile_pool(name="rope_rs", bufs=6) as rope_rs,
            tc.tile_pool(name="vsb", bufs=9) as vsb_pool,
            tc.tile_pool(name="attn", bufs=4) as attn_pool,
            tc.tile_pool(name="small", bufs=2) as small,
            tc.tile_pool(name="outsb", bufs=2) as outsb,
            tc.tile_pool(name="ps_qv", bufs=2, space="PSUM") as ps_qv,
            tc.tile_pool(name="ps_k", bufs=1, space="PSUM") as ps_k,
            tc.tile_pool(name="ps_sc", bufs=2, space="PSUM") as ps_sc,
            tc.tile_pool(name="ps_oc", bufs=2, space="PSUM") as ps_oc,
        ):
            # ---- persistent tiles ----
            xt = persist.tile([P, EC, T], F32R, tag="xt")
            cosq = persist.tile([P, S], F32R, tag="cosq")
            sinq = persist.tile([P, S], BF16, tag="sinq")
            cosk = persist.tile([P, T], F32R, tag="cosk")
            sink = persist.tile([P, T], BF16, tag="sink")
            masks = persist.tile([P, 4, 256], BF16, tag="masks")
            ones_full = persist.tile([P, P], BF16, tag="ones_full")
            out_norm = persist.tile([P, HEADS, S], F32R, tag="out_norm")

            def load_wq(h, name):
                w = wbig.tile([P, EC, HD], F32R, tag="wbig", name=name)
                for i8 in range(2):
                    nc.sync.dma_start(
                        w[:, 8 * i8 : 8 * i8 + 8, :],
                        (WQ if name[1] == "q" else WK).ap()[
                            h, :, 8 * i8 : 8 * i8 + 8, :
                        ],
                    )
                return w

            def load_wv(g):
                wv_qs = []
                for q in range(4):
                    wvq = wbig.tile([P, 4, 512], F32R, tag="wbig", name=f"wv{g}_{q}")
                    nc.sync.dma_start(
                        wvq[:, 0:2, :], WV.ap()[g, :, 4 * q : 4 * q + 2, :]
                    )
                    nc.sync.dma_start(
                        wvq[:, 2:4, :], WV.ap()[g, :, 4 * q + 2 : 4 * q + 4, :]
                    )
                    wv_qs.append(wvq)
                return wv_qs

            # ---- startup DMA order: xt interleaved with wq0/wk0 ----
            nc.sync.dma_start(cosq, COSQ.ap())
            wq0_sb = wbig.tile([P, EC, HD], F32R, tag="wbig", name="wq0")
            wk0_sb = wbig.tile([P, EC, HD], F32R, tag="wbig", name="wk0")
            for i4 in range(4):
                for ec in range(4 * i4, 4 * i4 + 4):
                    nc.sync.dma_start(xt[:, ec, :], XT.ap()[:, ec, :])
                nc.sync.dma_start(
                    wq0_sb[:, 4 * i4 : 4 * i4 + 4, :],
                    WQ.ap()[0, :, 4 * i4 : 4 * i4 + 4, :],
                )
                nc.sync.dma_start(
                    wk0_sb[:, 4 * i4 : 4 * i4 + 4, :],
                    WK.ap()[0, :, 4 * i4 : 4 * i4 + 4, :],
                )
            nc.sync.dma_start(sinq, SINQ.ap())
            nc.sync.dma_start(cosk, COSK.ap())
            nc.sync.dma_start(sink, SINK.ap())

            def emit_qk_compute(h, wq_sb, wk_sb, dummy_cb=None):
                # Q projection (feature-major [d, q]) + RoPE
                psq = ps_qv.tile([P, S], F32, tag="psqv", name=f"psq{h}")
                for ec in range(EC):
                    if dummy_cb is not None:
                        dummy_cb(ec)
                    nc.tensor.matmul(
                        psq,
                        wq_sb[:, ec, :],
                        xt[:, ec, WINDOW:T],
                        start=(ec == 0),
                        stop=(ec == EC - 1),
                    )
                q_sb = rope.tile([P, S], F32R, tag="qrope", name=f"q{h}")
                qraw_t = rope.tile([P, T], F32R, tag="raw", name=f"qraw{h}")
                qraw = qraw_t[:, 0:S]
                nc.scalar.copy(qraw, psq)
                nc.vector.tensor_mul(q_sb, psq, cosq)
                qsw_t = rope.tile([P, T], F32R, tag="sw", name=f"qsw{h}")
                qsw = qsw_t[:, 0:S]
                nc.sync.dma_start(qsw[0:64, :], qraw[64:128, :])
                nc.sync.dma_start(qsw[64:128, :], qraw[0:64, :])
                nc.gpsimd.tensor_mul(qsw, qsw, sinq)
                nc.vector.tensor_add(q_sb, q_sb, qsw)

                # K projection + RoPE (tokens 0..640 in two 320 halves)
                psk1 = ps_k.tile([P, HT], F32, tag="psk1", name=f"psk1_{h}")
                psk2 = ps_k.tile([P, HT], F32, tag="psk2", name=f"psk2_{h}")
                for ec in range(EC):
                    nc.tensor.matmul(
                        psk1,
                        wk_sb[:, ec, :],
                        xt[:, ec, 0:HT],
                        start=(ec == 0),
                        stop=(ec == EC - 1),
                    )
                for ec in range(EC):
                    nc.tensor.matmul(
                        psk2,
                        wk_sb[:, ec, :],
                        xt[:, ec, HT:T],
                        start=(ec == 0),
                        stop=(ec == EC - 1),
                    )
                k_sb = rope.tile([P, T], F32R, tag="krope", name=f"k{h}")
                kraw = rope.tile([P, T], F32R, tag="raw", name=f"kraw{h}")
                nc.scalar.copy(kraw[:, 0:HT], psk1)
                nc.scalar.copy(kraw[:, HT:T], psk2)
                nc.vector.tensor_mul(k_sb[:, 0:HT], psk1, cosk[:, 0:HT])
                nc.vector.tensor_mul(k_sb[:, HT:T], psk2, cosk[:, HT:T])
                ksw = rope.tile([P, T], F32R, tag="sw", name=f"ksw{h}")
                nc.sync.dma_start(ksw[0:64, :], kraw[64:128, :])
                nc.sync.dma_start(ksw[64:128, :], kraw[0:64, :])
                nc.gpsimd.tensor_mul(ksw, ksw, sink)
                nc.vector.tensor_add(k_sb, k_sb, ksw)
                return q_sb, k_sb

            def emit_proj(h):
                wq_sb = load_wq(h, f"wq{h}")
                wk_sb = load_wq(h, f"wk{h}")
                return emit_qk_compute(h, wq_sb, wk_sb)

            def emit_vproj_steady(g, wv_qs):
                # weights already prefetched a group ahead: tt-outer is fine
                v_tiles = []
                for tt in range(T // P):  # 5 token tiles
                    psv = ps_qv.tile([P, 512], F32, tag="psqv", name=f"psv{g}_{tt}")
                    for ec in range(EC):
                        nc.tensor.matmul(
                            psv,
                            xt[:, ec, tt * P : (tt + 1) * P],
                            wv_qs[ec // 4][:, ec % 4, :],
                            start=(ec == 0),
                            stop=(ec == EC - 1),
                        )
                    v_sb = vsb_pool.tile([P, 512], BF16, tag="vsb", name=f"v{g}_{tt}")
                    nc.scalar.copy(v_sb, psv)
                    v_tiles.append(v_sb)
                return v_tiles

            def emit_vproj_g0(wv_qs):
                # ec-outer: 5 parallel accumulators so V proceeds at DMA pace.
                # Borrow banks from the (still idle) attention pools.
                psv = [
                    ps_oc.tile([P, 512], F32, tag="oc", name="psv0_0"),
                    ps_oc.tile([P, 512], F32, tag="oc", name="psv0_1"),
                    ps_sc.tile([P, 512], F32, tag="sc", name="psv0_2"),
                    ps_sc.tile([P, 512], F32, tag="sc", name="psv0_3"),
                    ps_qv.tile([P, 512], F32, tag="psqv", name="psv0_4"),
                ]
                for ec in range(EC):
                    for tt in range(T // P):
                        nc.tensor.matmul(
                            psv[tt],
                            xt[:, ec, tt * P : (tt + 1) * P],
                            wv_qs[ec // 4][:, ec % 4, :],
                            start=(ec == 0),
                            stop=(ec == EC - 1),
                        )
                v_tiles = []
                for tt in range(T // P):
                    v_sb = vsb_pool.tile([P, 512], BF16, tag="vsb", name=f"v0_{tt}")
                    nc.scalar.copy(v_sb, psv[tt])
                    v_tiles.append(v_sb)
                return v_tiles

            def emit_attn(h, q_sb, k_sb, v_tiles):
                hh = h % GH
                for p in range(2):
                    qs = p * 256
                    ets = []
                    for j in range(3):  # roles R1,R2,R3 -> k-chunk 2p+j
                        c = 2 * p + j
                        midx = 3 if (j == 0 and p == 1) else j
                        psc = ps_sc.tile([P, 256], F32, tag="sc", name=f"sc{h}_{p}{j}")
                        nc.tensor.matmul(
                            psc,
                            k_sb[:, c * P : (c + 1) * P],
                            q_sb[:, qs : qs + 256],
                            start=True,
                            stop=True,
                        )
                        et = attn_pool.tile([P, 256], BF16, tag="attn", name=f"et{h}_{p}{j}")
                        nc.scalar.activation(
                            et, psc, mybir.ActivationFunctionType.Exp
                        )
                        nc.vector.tensor_mul(et, et, masks[:, midx, :])
                        ets.append(et)

                    poc = ps_oc.tile([P, 512], F32, tag="oc", name=f"poc{h}_{p}")
                    for j in range(3):
                        c = 2 * p + j
                        nc.tensor.matmul(
                            poc[:, 0:256],
                            v_tiles[c][:, hh * HD : (hh + 1) * HD],
                            ets[j],
                            start=(j == 0),
                            stop=False,
                        )
                        nc.tensor.matmul(
                            poc[:, 256:512],
                            ones_full,
                            ets[j],
                            start=False,
                            stop=(j == 2),
                        )
                    recip = small.tile([P, 256], F32, tag="recip", name=f"rc{h}_{p}")
                    nc.vector.reciprocal(recip, poc[:, 256:512])
                    nc.vector.tensor_mul(
                        out_norm[:, h, qs : qs + 256], poc[:, 0:256], recip
                    )

            def load_wo(eo):
                wo_qs = []
                for q in range(4):
                    woq = wbig.tile([P, 4, 512], F32R, tag="wbig", name=f"wo{eo}_{q}")
                    nc.sync.dma_start(
                        woq[:, 0:2, :], WO.ap()[eo, :, 4 * q : 4 * q + 2, :]
                    )
                    nc.sync.dma_start(
                        woq[:, 2:4, :], WO.ap()[eo, :, 4 * q + 2 : 4 * q + 4, :]
                    )
                    wo_qs.append(woq)
                return wo_qs

            # ---- weight-issue lookahead: head h's swaps are emitted AFTER
            # the weight loads for heads h+1/h+2, so the SP queue parking on
            # a data-dependent swap never delays already-issued prefetch. ----
            wv_g0 = load_wv(0)
            nc.sync.dma_start(masks, MASKS.ap().rearrange("m p q -> p m q"))
            nc.sync.dma_start(ones_full, ONES.ap())
            wtiles = {0: (wq0_sb, wk0_sb)}
            wtiles[1] = (load_wq(1, "wq1"), load_wq(1, "wk1"))
            wv_pending = {0: wv_g0}
            psd = ps_oc.tile([P, 512], F32, tag="oc", name="dummy_warm")
            dkeep = small.tile([1, 16], F32, tag="dkeep", name="dkeep")

            def warm_dummies(ec):
                for _ in range(4):
                    nc.tensor.matmul(
                        psd, cosq[:, 0:P], cosq, start=True, stop=True
                    )

            pending = None
            wo_qs = None
            for h in range(HEADS):
                # issue weights two heads ahead
                ahead = h + 2
                if ahead < HEADS:
                    if ahead % GH == 0:
                        wv_pending[ahead // GH] = load_wv(ahead // GH)
                    wtiles[ahead] = (
                        load_wq(ahead, f"wq{ahead}"),
                        load_wq(ahead, f"wk{ahead}"),
                    )
                elif ahead == HEADS:
                    wo_qs = load_wo(0)
                qk = emit_qk_compute(
                    h, *wtiles.pop(h), dummy_cb=(warm_dummies if h == 0 else None)
                )
                if h == 0:
                    # keep the dummy chain alive against DCE
                    nc.scalar.copy(dkeep, psd[0:1, 0:16])
                    nc.sync.dma_start(DBG.ap(), dkeep)
                if pending is not None:
                    ph = pending[0]
                    emit_attn(ph, pending[1], pending[2], v_groups[ph // GH])
                pending = (h, qk[0], qk[1])
                if h % GH == 0:
                    g = h // GH
                    if g == 0:
                        v_groups = {0: emit_vproj_g0(wv_pending.pop(0))}
                    else:
                        v_groups[g] = emit_vproj_steady(g, wv_pending.pop(g))

            ph = pending[0]
            emit_attn(ph, pending[1], pending[2], v_groups[ph // GH])

            # ---- out projection: OUT[t, e] = sum_hd out_norm^T . WO ----
            wo_tiles = {0: wo_qs}
            for eo in range(4):
                e0 = eo * 512
                if eo + 1 < 4:
                    wo_tiles[eo + 1] = load_wo(eo + 1)
                wo_qs = wo_tiles.pop(eo)
                for tt in range(4):
                    pso = ps_oc.tile([P, 512], F32, tag="oc")
                    for hd in range(HEADS):
                        nc.tensor.matmul(
                            pso,
                            out_norm[:, hd, tt * P : (tt + 1) * P],
                            wo_qs[hd // 4][:, hd % 4, :],
                            start=(hd == 0),
                            stop=(hd == HEADS - 1),
                        )
                    o_sb = outsb.tile([P, 512], F32, tag="osb")
                    nc.scalar.copy(o_sb, pso)
                    nc.scalar.dma_start(
                        OUT.ap()[tt * P : (tt + 1) * P, e0 : e0 + 512], o_sb
                    )

    if legalize:
        _legalize_single_wait(nc)
    return nc


def _rope_tables(pos, scale):
    """Feature-major [128, len(pos)] cos / sin' tables in de-interleaved d order.

    cos'[i, t] = cos(pos_t * invf[i % 64]) ; sin'[0:64] = -sin, sin'[64:128] = +sin.
    """
    inv_freq = 1.0 / (THETA ** (np.arange(0, HD, 2, dtype=np.float64) / HD))  # [64]
    ang = pos[None, :] * inv_freq[:, None]  # [64, T]
    cos = np.cos(ang)
    sin = np.sin(ang)
    cos_t = np.concatenate([cos, cos], axis=0) * scale
    sin_t = np.concatenate([-sin, sin], axis=0) * scale
    return cos_t.astype(np.float32), sin_t.astype(np.float32)


def _band_masks(start):
    """[4, 128, 256] multiplicative masks.

    Element (m, kp, qf): role m in {R1 pair0, R2, R3, R1 pair1};
    local key j = c*128 + kp, local query r = qs + qf;
    valid iff r <= j <= r + 128 and (global key) start - 128 + j >= 0.
    """
    out = np.zeros((4, P, 256), dtype=np.float32)
    roles = [(0, 0), (1, 0), (2, 0), (2, 256)]  # (chunk c, query offset qs)
    for m, (c, qs) in enumerate(roles):
        kp = np.arange(P)[:, None]
        qf = np.arange(256)[None, :]
        j = c * P + kp
        r = qs + qf
        valid = (r <= j) & (j <= r + WINDOW) & (start - WINDOW + j >= 0)
        out[m] = valid.astype(np.float32)
    return out


_CACHED = {}
LAST_RESULT = {}


def prepare_in_maps(x, W_qkv, W_out, b_out):
    x = np.asarray(x, dtype=np.float32)
    W_qkv = np.asarray(W_qkv, dtype=np.float32)
    W_out = np.asarray(W_out, dtype=np.float32)

    # host-side weight layout prep
    perm = np.concatenate([np.arange(0, HD, 2), np.arange(1, HD, 2)])  # de-interleave
    w4 = W_qkv.reshape(EMBED, HEADS, HD, 3)
    # [h, e, d] -> [h, p, ec, d] partition-major contiguous
    WQ = w4[..., 0].transpose(1, 0, 2)[:, :, perm].reshape(HEADS, EMBED // P, P, HD)
    WQ = np.ascontiguousarray(WQ.transpose(0, 2, 1, 3))
    WK = w4[..., 1].transpose(1, 0, 2)[:, :, perm].reshape(HEADS, EMBED // P, P, HD)
    WK = np.ascontiguousarray(WK.transpose(0, 2, 1, 3))
    # [e, f] -> [g, p, ec, 512]
    WV = w4[..., 2].reshape(EMBED // P, P, 4, 512)
    WV = np.ascontiguousarray(WV.transpose(2, 1, 0, 3))
    WOa = W_out.reshape(EMBED // P, P, 4, 512)
    WOa = np.ascontiguousarray(WOa.transpose(2, 1, 0, 3))

    in_maps = []
    for core in range(NCORES):
        b = core // 4
        start = (core % 4) * S
        # x^T with halo, zero-padded at the left for chunk 0
        xt = np.zeros((EMBED, T), dtype=np.float32)
        lo = start - WINDOW
        src = x[b, max(lo, 0) : start + S, :]  # [<=640, e]
        xt[:, T - src.shape[0] :] = src.T
        xt = np.ascontiguousarray(xt.reshape(EMBED // P, P, T).transpose(1, 0, 2))
        # rope tables: query positions start..start+512, key positions lo..start+512
        qpos = np.arange(start, start + S, dtype=np.float64)
        kpos = np.maximum(np.arange(lo, start + S, dtype=np.float64), 0.0)
        scale = 1.0 / math.sqrt(HD)
        cq, sq = _rope_tables(qpos, scale)
        ck, sk = _rope_tables(kpos, 1.0)
        in_maps.append(
            {
                "XT": xt,
                "WQ": WQ,
                "WK": WK,
                "WV": WV,
                "WO": WOa,
                "COSQ": cq,
                "SINQ": sq.astype(BF),
                "COSK": ck,
                "SINK": sk.astype(BF),
                "MASKS": _band_masks(start).astype(BF),
                "ONES": np.ones((P, P), dtype=BF),
            }
        )
    return in_maps


def kernel(x, W_qkv, W_out, b_out):
    in_maps = prepare_in_maps(x, W_qkv, W_out, b_out)
    b_out = np.asarray(b_out, dtype=np.float32)

    if "nc" not in _CACHED:
        _CACHED["nc"] = build_bass()
    nc = _CACHED["nc"]

    res = run_bass_kernel_spmd(nc, in_maps, core_ids=list(range(NCORES)))
    LAST_RESULT["res"] = res

    out = np.empty((B, L, EMBED), dtype=np.float32)
    for core in range(NCORES):
        b = core // 4
        start = (core % 4) * S
        out[b, start : start + S, :] = res.results[core]["OUT"] + b_out
    return out
